# revision 15
# baseline (speedup 1.0000x reference)
"""Trainium2 Bass kernel for CommonModule MHA (B=2, S=2048, V=64, H=16, D=1024).

Reference computation:
    u   = einsum('aibk,ajbk->aijk', qv, kv) / sqrt(V)     # [B,S,S,H]
    s   = softmax(u, axis=2)                               # over keys j
    att = einsum('aibk,abjk->aijk', s, vv)                 # [B,S,V,H]
    out = att.reshape(B,S,D) @ ov_w.T + ov_b               # [B,S,D]

Sharding: 8 cores = (batch a in {0,1}) x (query chunk ic in {0..3}, 512 rows
each).  Attention + projection are fully parallel over query rows, so each
core computes its full output slice with no cross-core reduction.

Per-core device algorithm (all in the "transposed" layout so that softmax
normalization can ride along the matmuls):
  - scores.T tile [128 j, 512 i] = K_h.T(lhsT) @ Q_h.T(rhs), contraction b=64
  - exp on ScalarE (scale=1/sqrt(V) folded into the activation), bf16 out
  - attU.T [65, 512] += V2_h(lhsT [128 j, 65]) @ expS(rhs), accumulated in
    PSUM over 16 j-tiles.  V2 carries an appended ones-row, so row 64 of the
    accumulator is the softmax denominator for free.
  - reciprocal of the denominator row, PE outer-product broadcast to
    [64, 512], one VectorE multiply -> normalized att.T
  - output projection computed transposed: outT [dout, i] accumulating over
    the 1024-dim contraction in 8 tiles of 128 (= head pairs; ov_w columns
    are host-permuted to head-major order to make att.T rows contiguous)
"""

import numpy as np

import concourse.bass as bass
import concourse.mybir as mybir
import concourse.tile as tile
from concourse import bacc
from concourse.bass_utils import run_bass_kernel_spmd

B, S, V, H = 2, 2048, 64, 16
D = V * H
IC = 512            # query rows per core
NJT = S // 128      # 16 j-tiles
NKT = D // 128      # 8 contraction tiles in the projection (= head pairs)
NDT = D // 128      # 8 output-dim tiles
SCALE = 1.0 / np.sqrt(V).astype(np.float32)
GRP = 2             # score j-tiles per ACT exp instruction (PSUM banks)

f32 = mybir.dt.float32
bf16 = mybir.dt.bfloat16


def build_program() -> bass.Bass:
    nc = bacc.Bacc()

    qT = nc.declare_dram_parameter("qT", [64, H, IC], bf16, isOutput=False)
    kT = nc.declare_dram_parameter("kT", [64, H, S], bf16, isOutput=False)
    v2 = nc.declare_dram_parameter("v2", [128, NJT, H, V + 1], bf16, isOutput=False)
    w2 = nc.declare_dram_parameter("w2", [128, NKT, D], bf16, isOutput=False)
    bi = nc.declare_dram_parameter("bi", [128, NDT], f32, isOutput=False)
    outT = nc.declare_dram_parameter("outT", [D, IC], f32, isOutput=True)

    with tile.TileContext(nc) as tc:
        with (
            tc.tile_pool(name="const", bufs=1) as const,
            tc.tile_pool(name="kpool", bufs=2) as kpool,
            tc.tile_pool(name="xpool", bufs=4) as xpool,
            tc.tile_pool(name="rpool", bufs=2) as rpool,
            tc.tile_pool(name="tpool", bufs=2) as tpool,
            tc.tile_pool(name="opool", bufs=2) as opool,
            tc.tile_pool(name="spsum", bufs=2, space="PSUM") as spsum,
            tc.tile_pool(name="apsum", bufs=2, space="PSUM") as apsum,
            tc.tile_pool(name="xpsum", bufs=2, space="PSUM") as xpsum,
        ):
            # ---- constants / big resident tiles ----
            qT_sb = const.tile([64, H, IC], bf16)
            v2_sb = const.tile([128, NJT, H, V + 1], bf16)
            w2_sb = const.tile([128, NKT, D], bf16)
            bi_sb = const.tile([128, NDT], f32)
            attT = const.tile([128, NKT, IC], bf16)
            ones_sb = const.tile([65, 64], f32)  # only row 64 used (as lhsT)
            wu_a = const.tile([128, 128], bf16)
            wu_b = const.tile([128, IC], bf16)

            # HAM warmup: ~20 dependency-free matmuls keep the PE busy for
            # >3.4us contiguously during the initial DMAs, lifting the clock
            # gate to 8/8 before the real pipeline starts (sub-us bubbles
            # later never re-throttle it).
            nc.vector.memset(wu_a, 0.0)
            nc.vector.memset(wu_b, 0.0)
            nc.vector.memset(ones_sb, 1.0)
            wups = spsum.tile([128, GRP, IC], f32, tag="sc", name="wups")
            for i in range(20):
                nc.tensor.matmul(
                    wups[:, i % GRP, :], lhsT=wu_a[:], rhs=wu_b[:],
                    start=True, stop=True,
                )

            for jt in range(NJT):
                nc.sync.dma_start(out=v2_sb[:, jt, :, :], in_=v2[:, jt, :, :])
            nc.sync.dma_start(out=w2_sb[:], in_=w2[:])
            nc.sync.dma_start(out=bi_sb[:], in_=bi[:])
            for h in range(H):
                nc.sync.dma_start(out=qT_sb[:, h, :], in_=qT[:, h, :])

            # ---- attention over heads ----
            kt_tiles = {}
            kt_tiles[0] = kpool.tile([64, S], bf16, tag="kt", name="kt0")
            nc.sync.dma_start(out=kt_tiles[0], in_=kT[:, 0, :])

            for h in range(H):
                if h + 1 < H:
                    kt_tiles[h + 1] = kpool.tile(
                        [64, S], bf16, tag="kt", name=f"kt{h + 1}"
                    )
                    nc.sync.dma_start(out=kt_tiles[h + 1], in_=kT[:, h + 1, :])
                k_sb = kt_tiles.pop(h)

                attU = apsum.tile([128, IC], f32, tag="attU")
                ngrp = (NJT + GRP - 1) // GRP
                for g in range(ngrp):
                    jts = range(g * GRP, min((g + 1) * GRP, NJT))
                    sc = spsum.tile([128, GRP, IC], f32, tag="sc")
                    for t, jt in enumerate(jts):
                        nc.tensor.matmul(
                            sc[:, t, :],
                            lhsT=k_sb[:, jt * 128 : (jt + 1) * 128],
                            rhs=qT_sb[:, h, :],
                            start=True,
                            stop=True,
                        )
                    ex = xpool.tile([128, GRP, IC], bf16, tag="ex")
                    nc.scalar.activation(
                        out=ex[:, 0 : len(jts), :],
                        in_=sc[:, 0 : len(jts), :],
                        func=mybir.ActivationFunctionType.Exp,
                        scale=float(SCALE),
                    )
                    for t, jt in enumerate(jts):
                        nc.tensor.matmul(
                            attU[0:65, :],
                            lhsT=v2_sb[:, jt, h, :],
                            rhs=ex[:, t, :],
                            start=(jt == 0),
                            stop=(jt == NJT - 1),
                        )

                # ---- per-head epilogue: denominator + normalize ----
                # Copy the raw denominator row out of PSUM (cheap), PE
                # outer-product broadcasts it to 64 partitions, reciprocal
                # runs wide on [64, IC] (a [1, IC] reciprocal costs 3.4us).
                den = rpool.tile([65, IC], f32, tag="den")
                nc.vector.tensor_copy(out=den[64:65, :], in_=attU[64:65, :])
                dbc = xpsum.tile([128, IC], f32, tag="dbc")
                nc.tensor.matmul(
                    dbc[0:64, :],
                    lhsT=ones_sb[64:65, :],
                    rhs=den[64:65, :],
                    start=True,
                    stop=True,
                )
                rbc_sb = rpool.tile([64, IC], f32, tag="rbc_sb")
                nc.vector.reciprocal_approx_fast(out=rbc_sb[:], in_=dbc[0:64, :])
                kt = h // 2
                if h % 2 == 0:
                    nc.vector.tensor_mul(
                        out=attT[0:64, kt, :], in0=attU[0:64, :], in1=rbc_sb[:]
                    )
                else:
                    tmp = tpool.tile([64, IC], bf16, tag="tmp")
                    nc.vector.tensor_mul(
                        out=tmp[:], in0=attU[0:64, :], in1=rbc_sb[:]
                    )
                    nc.sync.dma_start(out=attT[64:128, kt, :], in_=tmp[:])

            # ---- output projection (transposed): outT[dout, i] ----
            for dt in range(NDT):
                po = xpsum.tile([128, IC], f32, tag="dbc")
                for kt in range(NKT):
                    nc.tensor.matmul(
                        po[:],
                        lhsT=w2_sb[:, kt, dt * 128 : (dt + 1) * 128],
                        rhs=attT[:, kt, :],
                        start=(kt == 0),
                        stop=(kt == NKT - 1),
                    )
                ot = opool.tile([128, IC], f32, tag="ot")
                nc.vector.tensor_scalar_add(
                    out=ot[:], in0=po[:], scalar1=bi_sb[:, dt : dt + 1]
                )
                nc.sync.dma_start(out=outT[dt * 128 : (dt + 1) * 128, :], in_=ot[:])

    nc.compile()
    return nc


def make_core_inputs(qv, kv, vv, ov_w, ov_b):
    """Host-side sharding / relayout.  Returns list of 8 input maps."""
    qv = np.asarray(qv, dtype=np.float32)
    kv = np.asarray(kv, dtype=np.float32)
    vv = np.asarray(vv, dtype=np.float32)
    ov_w = np.asarray(ov_w, dtype=np.float32)
    ov_b = np.asarray(ov_b, dtype=np.float32)

    nbf = mybir.dt.np(bf16)
    # Projection weights, head-major permuted and transposed:
    #   w2[din_new, dout] = ov_w[dout, v*H + h]  with din_new = h*64 + v
    w2 = np.ascontiguousarray(
        ov_w.reshape(D, V, H).transpose(2, 1, 0).reshape(D, D)
    )
    w2_t = np.ascontiguousarray(
        w2.reshape(NKT, 128, D).transpose(1, 0, 2)
    ).astype(nbf)  # [128, NKT, D]
    bi_t = np.ascontiguousarray(ov_b.reshape(NDT, 128).T)  # [128, NDT]

    in_maps = []
    for a in range(B):
        kT_a = np.ascontiguousarray(kv[a].transpose(1, 2, 0)).astype(nbf)
        v2_a = np.empty((S, H, V + 1), dtype=np.float32)
        v2_a[:, :, :V] = vv[a].transpose(0, 2, 1)  # [S, H, V]
        v2_a[:, :, V] = 1.0
        v2_t = np.ascontiguousarray(
            v2_a.reshape(NJT, 128, H, V + 1).transpose(1, 0, 2, 3)
        ).astype(nbf)  # [128, NJT, H, V+1]
        for ic in range(4):
            qT_c = np.ascontiguousarray(
                qv[a, ic * IC : (ic + 1) * IC].transpose(1, 2, 0)
            ).astype(nbf)  # [64, H, IC]
            in_maps.append(
                {"qT": qT_c, "kT": kT_a, "v2": v2_t, "w2": w2_t, "bi": bi_t}
            )
    return in_maps


_PROGRAM_CACHE = []


def _get_program():
    if not _PROGRAM_CACHE:
        _PROGRAM_CACHE.append(build_program())
    return _PROGRAM_CACHE[0]


def run(inputs: dict, trace: bool = False):
    """Run on 8 cores; returns (full_output [B,S,D] f32, BassKernelResults)."""
    nc = _get_program()
    in_maps = make_core_inputs(**inputs)
    res = run_bass_kernel_spmd(nc, in_maps, core_ids=list(range(8)), trace=trace)
    out = np.empty((B, S, D), dtype=np.float32)
    for c in range(8):
        a, ic = c // 4, c % 4
        out[a, ic * IC : (ic + 1) * IC, :] = res.results[c]["outT"].T
    return out, res


def kernel(**inputs) -> np.ndarray:
    out, _ = run(inputs, trace=False)
    return out


# revision 17
# speedup vs baseline: 1.1817x; 1.1817x over previous
"""Trainium2 Bass kernel for CommonModule MHA (B=2, S=2048, V=64, H=16, D=1024).

Reference computation:
    u   = einsum('aibk,ajbk->aijk', qv, kv) / sqrt(V)     # [B,S,S,H]
    s   = softmax(u, axis=2)                               # over keys j
    att = einsum('aibk,abjk->aijk', s, vv)                 # [B,S,V,H]
    out = att.reshape(B,S,D) @ ov_w.T + ov_b               # [B,S,D]

Sharding: 8 cores = (batch a in {0,1}) x (query chunk ic in {0..3}, 512 rows
each).  Attention + projection are fully parallel over query rows, so each
core computes its full output slice with no cross-core reduction.

Per-core device algorithm (all in the "transposed" layout so that softmax
normalization can ride along the matmuls):
  - scores.T tile [128 j, 512 i] = K_h.T(lhsT) @ Q_h.T(rhs), contraction b=64
  - exp on ScalarE (scale=1/sqrt(V) folded into the activation), bf16 out
  - attU.T [65, 512] += V2_h(lhsT [128 j, 65]) @ expS(rhs), accumulated in
    PSUM over 16 j-tiles.  V2 carries an appended ones-row, so row 64 of the
    accumulator is the softmax denominator for free.
  - reciprocal of the denominator row, PE outer-product broadcast to
    [64, 512], one VectorE multiply -> normalized att.T
  - output projection computed transposed: outT [dout, i] accumulating over
    the 1024-dim contraction in 8 tiles of 128 (= head pairs; ov_w columns
    are host-permuted to head-major order to make att.T rows contiguous)
"""

import numpy as np

import concourse.bass as bass
import concourse.mybir as mybir
import concourse.tile as tile
from concourse import bacc
from concourse.bass_utils import run_bass_kernel_spmd

B, S, V, H = 2, 2048, 64, 16
D = V * H
IC = 512            # query rows per core
NJT = S // 128      # 16 j-tiles
NKT = D // 128      # 8 contraction tiles in the projection (= head pairs)
NDT = D // 128      # 8 output-dim tiles
SCALE = 1.0 / np.sqrt(V).astype(np.float32)
GRP = 2             # score j-tiles per ACT exp instruction (PSUM banks)

f32 = mybir.dt.float32
bf16 = mybir.dt.bfloat16


def build_program() -> bass.Bass:
    nc = bacc.Bacc()

    qT = nc.declare_dram_parameter("qT", [64, H, IC], bf16, isOutput=False)
    kT = nc.declare_dram_parameter("kT", [64, H, S], bf16, isOutput=False)
    v2 = nc.declare_dram_parameter("v2", [128, NJT, H, V + 1], bf16, isOutput=False)
    w2 = nc.declare_dram_parameter("w2", [128, NKT, D], bf16, isOutput=False)
    bi = nc.declare_dram_parameter("bi", [128, NDT], f32, isOutput=False)
    outT = nc.declare_dram_parameter("outT", [D, IC], f32, isOutput=True)

    with tile.TileContext(nc) as tc:
        with (
            tc.tile_pool(name="const", bufs=1) as const,
            tc.tile_pool(name="kpool", bufs=2) as kpool,
            tc.tile_pool(name="xpool", bufs=18) as xpool,
            tc.tile_pool(name="rpool", bufs=2) as rpool,
            tc.tile_pool(name="tpool", bufs=2) as tpool,
            tc.tile_pool(name="opool", bufs=2) as opool,
            tc.tile_pool(name="spsum", bufs=2, space="PSUM") as spsum,
            tc.tile_pool(name="apsum", bufs=2, space="PSUM") as apsum,
            tc.tile_pool(name="xpsum", bufs=2, space="PSUM") as xpsum,
        ):
            # ---- constants / big resident tiles ----
            qT_sb = const.tile([64, H, IC], bf16)
            v2_sb = const.tile([128, NJT, H, V + 1], bf16)
            w2_sb = const.tile([128, NKT, D], bf16)
            bi_sb = const.tile([128, NDT], f32)
            attT = const.tile([128, NKT, IC], bf16)
            ones_sb = const.tile([65, 64], f32)  # only row 64 used (as lhsT)
            wu_a = const.tile([128, 128], bf16)
            wu_b = const.tile([128, IC], bf16)

            # HAM warmup: ~20 dependency-free matmuls keep the PE busy for
            # >3.4us contiguously during the initial DMAs, lifting the clock
            # gate to 8/8 before the real pipeline starts (sub-us bubbles
            # later never re-throttle it).
            nc.vector.memset(wu_a, 0.0)
            nc.vector.memset(wu_b, 0.0)
            nc.vector.memset(ones_sb, 1.0)
            wups = spsum.tile([128, GRP, IC], f32, tag="sc", name="wups")
            for i in range(20):
                nc.tensor.matmul(
                    wups[:, i % GRP, :], lhsT=wu_a[:], rhs=wu_b[:],
                    start=True, stop=True,
                )

            # inputs needed first (scores of head 0) come first
            for h in range(H):
                nc.sync.dma_start(out=qT_sb[:, h, :], in_=qT[:, h, :])
            kt_tiles = {}
            kt_tiles[0] = kpool.tile([64, S], bf16, tag="kt", name="kt0")
            nc.sync.dma_start(out=kt_tiles[0], in_=kT[:, 0, :])
            for jt in range(NJT):
                nc.sync.dma_start(out=v2_sb[:, jt, :, :], in_=v2[:, jt, :, :])
            nc.sync.dma_start(out=w2_sb[:], in_=w2[:])
            nc.sync.dma_start(out=bi_sb[:], in_=bi[:])

            # ---- attention, software-pipelined with a 1-head skew ----
            # Slot s issues scores+exp for head s and attU for head s-1, so
            # every attU matmul consumes an exp tile finished a full head
            # earlier: the PE never blocks on ScalarE and stays dense enough
            # to hold the HAM clock gate at 8/8.
            NGRP = NJT // GRP
            exp_tiles = {}
            attUs = {}
            for s in range(H + 1):
                if s < H:
                    if s + 1 < H:
                        kt_tiles[s + 1] = kpool.tile(
                            [64, S], bf16, tag="kt", name=f"kt{s + 1}"
                        )
                        nc.sync.dma_start(out=kt_tiles[s + 1], in_=kT[:, s + 1, :])
                    k_sb = kt_tiles.pop(s)
                if s >= 1:
                    attUs[s - 1] = apsum.tile(
                        [128, IC], f32, tag="attU", name=f"attU{s - 1}"
                    )
                for g in range(NGRP):
                    if s < H:
                        sc = spsum.tile([128, GRP, IC], f32, tag="sc")
                        for t in range(GRP):
                            jt = g * GRP + t
                            nc.tensor.matmul(
                                sc[:, t, :],
                                lhsT=k_sb[:, jt * 128 : (jt + 1) * 128],
                                rhs=qT_sb[:, s, :],
                                start=True,
                                stop=True,
                            )
                        ex = xpool.tile(
                            [128, GRP, IC], bf16, tag="ex", name=f"ex{s}_{g}"
                        )
                        nc.scalar.activation(
                            out=ex[:],
                            in_=sc[:],
                            func=mybir.ActivationFunctionType.Exp,
                            scale=float(SCALE),
                        )
                        exp_tiles[(s, g)] = ex
                    if s >= 1:
                        h = s - 1
                        exp_prev = exp_tiles.pop((h, g))
                        for t in range(GRP):
                            jt = g * GRP + t
                            nc.tensor.matmul(
                                attUs[h][0:65, :],
                                lhsT=v2_sb[:, jt, h, :],
                                rhs=exp_prev[:, t, :],
                                start=(jt == 0),
                                stop=(jt == NJT - 1),
                            )
                if s >= 1:
                    # ---- epilogue for head s-1: denominator + normalize ----
                    # Copy the raw denominator row out of PSUM (cheap), PE
                    # outer-product broadcasts it to 64 partitions, then the
                    # reciprocal runs wide on [64, IC] (a [1, IC] reciprocal
                    # costs 3.4us serial).
                    h = s - 1
                    attU = attUs.pop(h)
                    den = rpool.tile([65, IC], f32, tag="den")
                    nc.vector.tensor_copy(out=den[64:65, :], in_=attU[64:65, :])
                    dbc = xpsum.tile([128, IC], f32, tag="dbc")
                    nc.tensor.matmul(
                        dbc[0:64, :],
                        lhsT=ones_sb[64:65, :],
                        rhs=den[64:65, :],
                        start=True,
                        stop=True,
                    )
                    rbc_sb = rpool.tile([64, IC], f32, tag="rbc_sb")
                    nc.vector.reciprocal_approx_fast(out=rbc_sb[:], in_=dbc[0:64, :])
                    kt = h // 2
                    if h % 2 == 0:
                        nc.vector.tensor_mul(
                            out=attT[0:64, kt, :], in0=attU[0:64, :], in1=rbc_sb[:]
                        )
                    else:
                        tmp = tpool.tile([64, IC], bf16, tag="tmp")
                        nc.vector.tensor_mul(
                            out=tmp[:], in0=attU[0:64, :], in1=rbc_sb[:]
                        )
                        nc.sync.dma_start(out=attT[64:128, kt, :], in_=tmp[:])

            # ---- output projection (transposed): outT[dout, i] ----
            for dt in range(NDT):
                po = xpsum.tile([128, IC], f32, tag="dbc")
                for kt in range(NKT):
                    nc.tensor.matmul(
                        po[:],
                        lhsT=w2_sb[:, kt, dt * 128 : (dt + 1) * 128],
                        rhs=attT[:, kt, :],
                        start=(kt == 0),
                        stop=(kt == NKT - 1),
                    )
                ot = opool.tile([128, IC], f32, tag="ot")
                nc.vector.tensor_scalar_add(
                    out=ot[:], in0=po[:], scalar1=bi_sb[:, dt : dt + 1]
                )
                nc.sync.dma_start(out=outT[dt * 128 : (dt + 1) * 128, :], in_=ot[:])

    nc.compile()
    return nc


def make_core_inputs(qv, kv, vv, ov_w, ov_b):
    """Host-side sharding / relayout.  Returns list of 8 input maps."""
    qv = np.asarray(qv, dtype=np.float32)
    kv = np.asarray(kv, dtype=np.float32)
    vv = np.asarray(vv, dtype=np.float32)
    ov_w = np.asarray(ov_w, dtype=np.float32)
    ov_b = np.asarray(ov_b, dtype=np.float32)

    nbf = mybir.dt.np(bf16)
    # Projection weights, head-major permuted and transposed:
    #   w2[din_new, dout] = ov_w[dout, v*H + h]  with din_new = h*64 + v
    w2 = np.ascontiguousarray(
        ov_w.reshape(D, V, H).transpose(2, 1, 0).reshape(D, D)
    )
    w2_t = np.ascontiguousarray(
        w2.reshape(NKT, 128, D).transpose(1, 0, 2)
    ).astype(nbf)  # [128, NKT, D]
    bi_t = np.ascontiguousarray(ov_b.reshape(NDT, 128).T)  # [128, NDT]

    in_maps = []
    for a in range(B):
        kT_a = np.ascontiguousarray(kv[a].transpose(1, 2, 0)).astype(nbf)
        v2_a = np.empty((S, H, V + 1), dtype=np.float32)
        v2_a[:, :, :V] = vv[a].transpose(0, 2, 1)  # [S, H, V]
        v2_a[:, :, V] = 1.0
        v2_t = np.ascontiguousarray(
            v2_a.reshape(NJT, 128, H, V + 1).transpose(1, 0, 2, 3)
        ).astype(nbf)  # [128, NJT, H, V+1]
        for ic in range(4):
            qT_c = np.ascontiguousarray(
                qv[a, ic * IC : (ic + 1) * IC].transpose(1, 2, 0)
            ).astype(nbf)  # [64, H, IC]
            in_maps.append(
                {"qT": qT_c, "kT": kT_a, "v2": v2_t, "w2": w2_t, "bi": bi_t}
            )
    return in_maps


_PROGRAM_CACHE = []


def _get_program():
    if not _PROGRAM_CACHE:
        _PROGRAM_CACHE.append(build_program())
    return _PROGRAM_CACHE[0]


def run(inputs: dict, trace: bool = False):
    """Run on 8 cores; returns (full_output [B,S,D] f32, BassKernelResults)."""
    nc = _get_program()
    in_maps = make_core_inputs(**inputs)
    res = run_bass_kernel_spmd(nc, in_maps, core_ids=list(range(8)), trace=trace)
    out = np.empty((B, S, D), dtype=np.float32)
    for c in range(8):
        a, ic = c // 4, c % 4
        out[a, ic * IC : (ic + 1) * IC, :] = res.results[c]["outT"].T
    return out, res


def kernel(**inputs) -> np.ndarray:
    out, _ = run(inputs, trace=False)
    return out


# revision 19
# speedup vs baseline: 1.7846x; 1.5101x over previous
"""Trainium2 Bass kernel for CommonModule MHA (B=2, S=2048, V=64, H=16, D=1024).

Reference computation:
    u   = einsum('aibk,ajbk->aijk', qv, kv) / sqrt(V)     # [B,S,S,H]
    s   = softmax(u, axis=2)                               # over keys j
    att = einsum('aibk,abjk->aijk', s, vv)                 # [B,S,V,H]
    out = att.reshape(B,S,D) @ ov_w.T + ov_b               # [B,S,D]

Sharding: 8 cores = (batch a in {0,1}) x (query chunk ic in {0..3}, 512 rows
each).  Attention + projection are fully parallel over query rows, so each
core computes its full output slice with no cross-core reduction.

Per-core device algorithm (all in the "transposed" layout so that softmax
normalization can ride along the matmuls):
  - scores.T tile [128 j, 512 i] = K_h.T(lhsT) @ Q_h.T(rhs), contraction b=64
  - exp on ScalarE (scale=1/sqrt(V) folded into the activation), bf16 out
  - attU.T [65, 512] += V2_h(lhsT [128 j, 65]) @ expS(rhs), accumulated in
    PSUM over 16 j-tiles.  V2 carries an appended ones-row, so row 64 of the
    accumulator is the softmax denominator for free.
  - reciprocal of the denominator row, PE outer-product broadcast to
    [64, 512], one VectorE multiply -> normalized att.T
  - output projection computed transposed: outT [dout, i] accumulating over
    the 1024-dim contraction in 8 tiles of 128 (= head pairs; ov_w columns
    are host-permuted to head-major order to make att.T rows contiguous)
"""

import numpy as np

import concourse.bass as bass
import concourse.mybir as mybir
import concourse.tile as tile
from concourse import bacc
from concourse.bass_utils import run_bass_kernel_spmd

B, S, V, H = 2, 2048, 64, 16
D = V * H
IC = 512            # query rows per core
NJT = S // 128      # 16 j-tiles
NKT = D // 128      # 8 contraction tiles in the projection (= head pairs)
NDT = D // 128      # 8 output-dim tiles
SCALE = 1.0 / np.sqrt(V).astype(np.float32)
GRP = 2             # score j-tiles per ACT exp instruction (PSUM banks)

f32 = mybir.dt.float32
bf16 = mybir.dt.bfloat16


def build_program() -> bass.Bass:
    nc = bacc.Bacc()

    qT = nc.declare_dram_parameter("qT", [128, H, IC], bf16, isOutput=False)
    kT = nc.declare_dram_parameter("kT", [128, H, S], bf16, isOutput=False)
    v2 = nc.declare_dram_parameter("v2", [128, NJT, H, V + 1], bf16, isOutput=False)
    w2 = nc.declare_dram_parameter("w2", [128, NKT, D], bf16, isOutput=False)
    bi = nc.declare_dram_parameter("bi", [128, NDT], f32, isOutput=False)
    outT = nc.declare_dram_parameter("outT", [D, IC], f32, isOutput=True)

    with tile.TileContext(nc) as tc:
        with (
            tc.tile_pool(name="const", bufs=1) as const,
            tc.tile_pool(name="kpool", bufs=2) as kpool,
            tc.tile_pool(name="xpool", bufs=18) as xpool,
            tc.tile_pool(name="rpool", bufs=2) as rpool,
            tc.tile_pool(name="tpool", bufs=2) as tpool,
            tc.tile_pool(name="opool", bufs=2) as opool,
            tc.tile_pool(name="spsum", bufs=2, space="PSUM") as spsum,
            tc.tile_pool(name="apsum", bufs=2, space="PSUM") as apsum,
            tc.tile_pool(name="xpsum", bufs=2, space="PSUM") as xpsum,
        ):
            # ---- constants / big resident tiles ----
            qT_sb = const.tile([128, H, IC], bf16)
            v2_sb = const.tile([128, NJT, H, V + 1], bf16)
            w2_sb = const.tile([128, NKT, D], bf16)
            bi_sb = const.tile([128, NDT], f32)
            attT = const.tile([128, NKT, IC], bf16)
            ones_sb = const.tile([65, 64], f32)  # only row 64 used (as lhsT)
            wu_a = const.tile([128, 128], bf16)
            wu_b = const.tile([128, IC], bf16)

            # HAM warmup: ~20 dependency-free matmuls keep the PE busy for
            # >3.4us contiguously during the initial DMAs, lifting the clock
            # gate to 8/8 before the real pipeline starts (sub-us bubbles
            # later never re-throttle it).
            nc.vector.memset(wu_a, 0.0)
            nc.vector.memset(wu_b, 0.0)
            nc.vector.memset(ones_sb, 1.0)
            wups = spsum.tile([128, GRP, IC], f32, tag="sc", name="wups")
            for i in range(44):
                nc.tensor.matmul(
                    wups[:, i % GRP, :], lhsT=wu_a[:], rhs=wu_b[:],
                    start=True, stop=True,
                )

            # inputs needed first (scores of head 0) come first
            for h in range(H):
                nc.sync.dma_start(out=qT_sb[:, h, :], in_=qT[:, h, :])
            kt_tiles = {}
            kt_tiles[0] = kpool.tile([128, S], bf16, tag="kt", name="kt0")
            nc.sync.dma_start(out=kt_tiles[0], in_=kT[:, 0, :])
            for jt in range(NJT):
                nc.sync.dma_start(out=v2_sb[:, jt, :, :], in_=v2[:, jt, :, :])
            nc.sync.dma_start(out=w2_sb[:], in_=w2[:])
            nc.sync.dma_start(out=bi_sb[:], in_=bi[:])

            # ---- attention, software-pipelined with a 1-head skew ----
            # Slot s issues scores+exp for head s and attU for head s-1, so
            # every attU matmul consumes an exp tile finished a full head
            # earlier: the PE never blocks on ScalarE and stays dense enough
            # to hold the HAM clock gate at 8/8.
            NGRP = NJT // GRP
            exp_tiles = {}
            attUs = {}
            for s in range(H + 1):
                if s < H:
                    if s + 1 < H:
                        kt_tiles[s + 1] = kpool.tile(
                            [128, S], bf16, tag="kt", name=f"kt{s + 1}"
                        )
                        nc.sync.dma_start(out=kt_tiles[s + 1], in_=kT[:, s + 1, :])
                    k_sb = kt_tiles.pop(s)
                if s >= 1:
                    attUs[s - 1] = apsum.tile(
                        [128, IC], f32, tag="attU", name=f"attU{s - 1}"
                    )
                for g in range(NGRP):
                    if s < H:
                        sc = spsum.tile([128, GRP, IC], f32, tag="sc")
                        for t in range(GRP):
                            jt = g * GRP + t
                            nc.tensor.matmul(
                                sc[:, t, :],
                                lhsT=k_sb[:, jt * 128 : (jt + 1) * 128],
                                rhs=qT_sb[:, s, :],
                                start=True,
                                stop=True,
                            )
                        ex = xpool.tile(
                            [128, GRP, IC], bf16, tag="ex", name=f"ex{s}_{g}"
                        )
                        nc.scalar.activation(
                            out=ex[:],
                            in_=sc[:],
                            func=mybir.ActivationFunctionType.Exp,
                            scale=float(SCALE),
                        )
                        exp_tiles[(s, g)] = ex
                    if s >= 1:
                        h = s - 1
                        exp_prev = exp_tiles.pop((h, g))
                        for t in range(GRP):
                            jt = g * GRP + t
                            nc.tensor.matmul(
                                attUs[h][0:65, :],
                                lhsT=v2_sb[:, jt, h, :],
                                rhs=exp_prev[:, t, :],
                                start=(jt == 0),
                                stop=(jt == NJT - 1),
                            )
                if s >= 1:
                    # ---- epilogue for head s-1: denominator + normalize ----
                    # Copy the raw denominator row out of PSUM (cheap), PE
                    # outer-product broadcasts it to 64 partitions, then the
                    # reciprocal runs wide on [64, IC] (a [1, IC] reciprocal
                    # costs 3.4us serial).
                    h = s - 1
                    attU = attUs.pop(h)
                    den = rpool.tile([65, IC], f32, tag="den")
                    nc.vector.tensor_copy(out=den[64:65, :], in_=attU[64:65, :])
                    dbc = xpsum.tile([128, IC], f32, tag="dbc")
                    nc.tensor.matmul(
                        dbc[0:64, :],
                        lhsT=ones_sb[64:65, :],
                        rhs=den[64:65, :],
                        start=True,
                        stop=True,
                    )
                    rbc_sb = rpool.tile([64, IC], f32, tag="rbc_sb")
                    nc.vector.reciprocal_approx_fast(out=rbc_sb[:], in_=dbc[0:64, :])
                    kt = h // 2
                    if h % 2 == 0:
                        nc.vector.tensor_mul(
                            out=attT[0:64, kt, :], in0=attU[0:64, :], in1=rbc_sb[:]
                        )
                    else:
                        tmp = tpool.tile([64, IC], bf16, tag="tmp")
                        nc.vector.tensor_mul(
                            out=tmp[:], in0=attU[0:64, :], in1=rbc_sb[:]
                        )
                        nc.sync.dma_start(out=attT[64:128, kt, :], in_=tmp[:])

            # ---- output projection (transposed): outT[dout, i] ----
            for dt in range(NDT):
                po = xpsum.tile([128, IC], f32, tag="dbc")
                for kt in range(NKT):
                    nc.tensor.matmul(
                        po[:],
                        lhsT=w2_sb[:, kt, dt * 128 : (dt + 1) * 128],
                        rhs=attT[:, kt, :],
                        start=(kt == 0),
                        stop=(kt == NKT - 1),
                    )
                ot = opool.tile([128, IC], f32, tag="ot")
                nc.vector.tensor_scalar_add(
                    out=ot[:], in0=po[:], scalar1=bi_sb[:, dt : dt + 1]
                )
                nc.sync.dma_start(out=outT[dt * 128 : (dt + 1) * 128, :], in_=ot[:])

    nc.compile()
    return nc


def make_core_inputs(qv, kv, vv, ov_w, ov_b):
    """Host-side sharding / relayout.  Returns list of 8 input maps."""
    qv = np.asarray(qv, dtype=np.float32)
    kv = np.asarray(kv, dtype=np.float32)
    vv = np.asarray(vv, dtype=np.float32)
    ov_w = np.asarray(ov_w, dtype=np.float32)
    ov_b = np.asarray(ov_b, dtype=np.float32)

    nbf = mybir.dt.np(bf16)
    # Projection weights, head-major permuted and transposed:
    #   w2[din_new, dout] = ov_w[dout, v*H + h]  with din_new = h*64 + v
    w2 = np.ascontiguousarray(
        ov_w.reshape(D, V, H).transpose(2, 1, 0).reshape(D, D)
    )
    w2_t = np.ascontiguousarray(
        w2.reshape(NKT, 128, D).transpose(1, 0, 2)
    ).astype(nbf)  # [128, NKT, D]
    bi_t = np.ascontiguousarray(ov_b.reshape(NDT, 128).T)  # [128, NDT]

    in_maps = []
    for a in range(B):
        # contraction zero-padded from 64 to 128 rows (exact same scores;
        # keeps the PE array fully row-occupied)
        kT_a = np.zeros((128, H, S), dtype=nbf)
        kT_a[:V] = kv[a].transpose(1, 2, 0).astype(nbf)
        v2_a = np.empty((S, H, V + 1), dtype=np.float32)
        v2_a[:, :, :V] = vv[a].transpose(0, 2, 1)  # [S, H, V]
        v2_a[:, :, V] = 1.0
        v2_t = np.ascontiguousarray(
            v2_a.reshape(NJT, 128, H, V + 1).transpose(1, 0, 2, 3)
        ).astype(nbf)  # [128, NJT, H, V+1]
        for ic in range(4):
            qT_c = np.zeros((128, H, IC), dtype=nbf)
            qT_c[:V] = qv[a, ic * IC : (ic + 1) * IC].transpose(1, 2, 0).astype(nbf)
            in_maps.append(
                {"qT": qT_c, "kT": kT_a, "v2": v2_t, "w2": w2_t, "bi": bi_t}
            )
    return in_maps


_PROGRAM_CACHE = []


def _get_program():
    if not _PROGRAM_CACHE:
        _PROGRAM_CACHE.append(build_program())
    return _PROGRAM_CACHE[0]


def run(inputs: dict, trace: bool = False):
    """Run on 8 cores; returns (full_output [B,S,D] f32, BassKernelResults)."""
    nc = _get_program()
    in_maps = make_core_inputs(**inputs)
    res = run_bass_kernel_spmd(nc, in_maps, core_ids=list(range(8)), trace=trace)
    out = np.empty((B, S, D), dtype=np.float32)
    for c in range(8):
        a, ic = c // 4, c % 4
        out[a, ic * IC : (ic + 1) * IC, :] = res.results[c]["outT"].T
    return out, res


def kernel(**inputs) -> np.ndarray:
    out, _ = run(inputs, trace=False)
    return out


# revision 22
# speedup vs baseline: 1.8868x; 1.0573x over previous
"""Trainium2 Bass kernel for CommonModule MHA (B=2, S=2048, V=64, H=16, D=1024).

Reference computation:
    u   = einsum('aibk,ajbk->aijk', qv, kv) / sqrt(V)     # [B,S,S,H]
    s   = softmax(u, axis=2)                               # over keys j
    att = einsum('aibk,abjk->aijk', s, vv)                 # [B,S,V,H]
    out = att.reshape(B,S,D) @ ov_w.T + ov_b               # [B,S,D]

Sharding: 8 cores = (batch a in {0,1}) x (query chunk ic in {0..3}, 512 rows
each).  Attention + projection are fully parallel over query rows, so each
core computes its full output slice with no cross-core reduction.

Per-core device algorithm (all in the "transposed" layout so that softmax
normalization can ride along the matmuls):
  - scores.T tile [128 j, 512 i] = K_h.T(lhsT) @ Q_h.T(rhs), contraction b=64
  - exp on ScalarE (scale=1/sqrt(V) folded into the activation), bf16 out
  - attU.T [65, 512] += V2_h(lhsT [128 j, 65]) @ expS(rhs), accumulated in
    PSUM over 16 j-tiles.  V2 carries an appended ones-row, so row 64 of the
    accumulator is the softmax denominator for free.
  - reciprocal of the denominator row, PE outer-product broadcast to
    [64, 512], one VectorE multiply -> normalized att.T
  - output projection computed transposed: outT [dout, i] accumulating over
    the 1024-dim contraction in 8 tiles of 128 (= head pairs; ov_w columns
    are host-permuted to head-major order to make att.T rows contiguous)
"""

import numpy as np

import concourse.bass as bass
import concourse.mybir as mybir
import concourse.tile as tile
from concourse import bacc
from concourse.bass_utils import run_bass_kernel_spmd

B, S, V, H = 2, 2048, 64, 16
D = V * H
IC = 512            # query rows per core
NJT = S // 128      # 16 j-tiles
NKT = D // 128      # 8 contraction tiles in the projection (= head pairs)
NDT = D // 128      # 8 output-dim tiles
SCALE = 1.0 / np.sqrt(V).astype(np.float32)
GRP = 2             # score j-tiles per ACT exp instruction (PSUM banks)

f32 = mybir.dt.float32
f32r = mybir.dt.float32r
bf16 = mybir.dt.bfloat16


def build_program() -> bass.Bass:
    nc = bacc.Bacc()

    qT = nc.declare_dram_parameter("qT", [128, H, IC], bf16, isOutput=False)
    kT = nc.declare_dram_parameter("kT", [128, H, S], bf16, isOutput=False)
    v2 = nc.declare_dram_parameter("v2", [128, NJT, H, V + 1], bf16, isOutput=False)
    w2 = nc.declare_dram_parameter("w2", [128, NKT, D], bf16, isOutput=False)
    bi = nc.declare_dram_parameter("bi", [128, NDT], f32, isOutput=False)
    outT = nc.declare_dram_parameter("outT", [D, IC], f32, isOutput=True)

    with tile.TileContext(nc) as tc:
        with (
            tc.tile_pool(name="const", bufs=1) as const,
            tc.tile_pool(name="kpool", bufs=3) as kpool,
            tc.tile_pool(name="xpool", bufs=18) as xpool,
            tc.tile_pool(name="rpool", bufs=2) as rpool,
            tc.tile_pool(name="tpool", bufs=2) as tpool,
            tc.tile_pool(name="opool", bufs=2) as opool,
            tc.tile_pool(name="spsum", bufs=2, space="PSUM") as spsum,
            tc.tile_pool(name="apsum", bufs=2, space="PSUM") as apsum,
            tc.tile_pool(name="xpsum", bufs=2, space="PSUM") as xpsum,
        ):
            # ---- constants / big resident tiles ----
            qT_sb = const.tile([128, H, IC], bf16)
            v2_sb = const.tile([128, NJT, H, V + 1], bf16)
            w2_sb = const.tile([128, NKT, D], bf16)
            bi_sb = const.tile([128, NDT], f32)
            attT = const.tile([128, NKT, IC], bf16)
            ones_sb = const.tile([65, 64], f32)  # only row 64 used (as lhsT)
            wu_a = const.tile([128, 128], bf16)
            wu_b = const.tile([128, IC], bf16)

            # HAM warmup: ~20 dependency-free matmuls keep the PE busy for
            # >3.4us contiguously during the initial DMAs, lifting the clock
            # gate to 8/8 before the real pipeline starts (sub-us bubbles
            # later never re-throttle it).
            nc.vector.memset(wu_a, 0.0)
            nc.vector.memset(wu_b, 0.0)
            nc.vector.memset(ones_sb, 1.0)
            nc.vector.tensor_copy(
                out=ones_sb[64:65, :].bitcast(f32r), in_=ones_sb[64:65, :]
            )
            wups = spsum.tile([128, GRP, IC], f32, tag="sc", name="wups")
            for i in range(44):
                nc.tensor.matmul(
                    wups[:, i % GRP, :], lhsT=wu_a[:], rhs=wu_b[:],
                    start=True, stop=True,
                )

            # inputs needed first (scores of head 0) come first
            for h in range(H):
                nc.sync.dma_start(out=qT_sb[:, h, :], in_=qT[:, h, :])
            kt_tiles = {}
            for hh in (0, 1):
                kt_tiles[hh] = kpool.tile([128, S], bf16, tag="kt", name=f"kt{hh}")
                nc.sync.dma_start(out=kt_tiles[hh], in_=kT[:, hh, :])
            for jt in range(NJT):
                nc.sync.dma_start(out=v2_sb[:, jt, :, :], in_=v2[:, jt, :, :])
            nc.sync.dma_start(out=w2_sb[:], in_=w2[:])
            nc.sync.dma_start(out=bi_sb[:], in_=bi[:])

            # ---- attention, software-pipelined with a 1-head skew ----
            # Slot s issues scores+exp for head s and attU for head s-1, so
            # every attU matmul consumes an exp tile finished a full head
            # earlier: the PE never blocks on ScalarE and stays dense enough
            # to hold the HAM clock gate at 8/8.
            NGRP = NJT // GRP
            exp_tiles = {}
            attUs = {}
            for s in range(H + 1):
                if s < H:
                    if s + 2 < H:
                        kt_tiles[s + 2] = kpool.tile(
                            [128, S], bf16, tag="kt", name=f"kt{s + 2}"
                        )
                        nc.sync.dma_start(out=kt_tiles[s + 2], in_=kT[:, s + 2, :])
                    k_sb = kt_tiles.pop(s)
                if s >= 1:
                    attUs[s - 1] = apsum.tile(
                        [128, IC], f32, tag="attU", name=f"attU{s - 1}"
                    )
                for g in range(NGRP):
                    if s < H:
                        sc = spsum.tile([128, GRP, IC], f32, tag="sc")
                        for t in range(GRP):
                            jt = g * GRP + t
                            nc.tensor.matmul(
                                sc[:, t, :],
                                lhsT=k_sb[:, jt * 128 : (jt + 1) * 128],
                                rhs=qT_sb[:, s, :],
                                start=True,
                                stop=True,
                            )
                        ex = xpool.tile(
                            [128, GRP, IC], bf16, tag="ex", name=f"ex{s}_{g}"
                        )
                        nc.scalar.activation(
                            out=ex[:],
                            in_=sc[:],
                            func=mybir.ActivationFunctionType.Exp,
                            scale=float(SCALE),
                        )
                        exp_tiles[(s, g)] = ex
                    if s >= 1:
                        h = s - 1
                        exp_prev = exp_tiles.pop((h, g))
                        for t in range(GRP):
                            jt = g * GRP + t
                            nc.tensor.matmul(
                                attUs[h][0:65, :],
                                lhsT=v2_sb[:, jt, h, :],
                                rhs=exp_prev[:, t, :],
                                start=(jt == 0),
                                stop=(jt == NJT - 1),
                            )
                if s >= 1:
                    # ---- epilogue for head s-1: denominator + normalize ----
                    # Copy the raw denominator row out of PSUM (cheap), PE
                    # outer-product broadcasts it to 64 partitions, then the
                    # reciprocal runs wide on [64, IC] (a [1, IC] reciprocal
                    # costs 3.4us serial).
                    h = s - 1
                    attU = attUs.pop(h)
                    den = rpool.tile([65, IC], f32, tag="den")
                    nc.vector.tensor_copy(out=den[64:65, :].bitcast(f32r), in_=attU[64:65, :])
                    dbc = xpsum.tile([128, IC], f32, tag="dbc")
                    nc.tensor.matmul(
                        dbc[0:64, :],
                        lhsT=ones_sb[64:65, :].bitcast(f32r),
                        rhs=den[64:65, :].bitcast(f32r),
                        start=True,
                        stop=True,
                    )
                    rbc_sb = rpool.tile([64, IC], f32, tag="rbc_sb")
                    nc.vector.reciprocal_approx_fast(out=rbc_sb[:], in_=dbc[0:64, :])
                    kt = h // 2
                    if h % 2 == 0:
                        nc.vector.tensor_mul(
                            out=attT[0:64, kt, :], in0=attU[0:64, :], in1=rbc_sb[:]
                        )
                    else:
                        tmp = tpool.tile([64, IC], bf16, tag="tmp")
                        nc.vector.tensor_mul(
                            out=tmp[:], in0=attU[0:64, :], in1=rbc_sb[:]
                        )
                        nc.sync.dma_start(out=attT[64:128, kt, :], in_=tmp[:])

            # ---- output projection (transposed): outT[dout, i] ----
            for dt in range(NDT):
                po = xpsum.tile([128, IC], f32, tag="dbc")
                for kt in range(NKT):
                    nc.tensor.matmul(
                        po[:],
                        lhsT=w2_sb[:, kt, dt * 128 : (dt + 1) * 128],
                        rhs=attT[:, kt, :],
                        start=(kt == 0),
                        stop=(kt == NKT - 1),
                    )
                ot = opool.tile([128, IC], f32, tag="ot")
                nc.vector.tensor_scalar_add(
                    out=ot[:], in0=po[:], scalar1=bi_sb[:, dt : dt + 1]
                )
                nc.sync.dma_start(out=outT[dt * 128 : (dt + 1) * 128, :], in_=ot[:])

    nc.compile()
    return nc


def make_core_inputs(qv, kv, vv, ov_w, ov_b):
    """Host-side sharding / relayout.  Returns list of 8 input maps."""
    qv = np.asarray(qv, dtype=np.float32)
    kv = np.asarray(kv, dtype=np.float32)
    vv = np.asarray(vv, dtype=np.float32)
    ov_w = np.asarray(ov_w, dtype=np.float32)
    ov_b = np.asarray(ov_b, dtype=np.float32)

    nbf = mybir.dt.np(bf16)
    # Projection weights, head-major permuted and transposed:
    #   w2[din_new, dout] = ov_w[dout, v*H + h]  with din_new = h*64 + v
    w2 = np.ascontiguousarray(
        ov_w.reshape(D, V, H).transpose(2, 1, 0).reshape(D, D)
    )
    w2_t = np.ascontiguousarray(
        w2.reshape(NKT, 128, D).transpose(1, 0, 2)
    ).astype(nbf)  # [128, NKT, D]
    bi_t = np.ascontiguousarray(ov_b.reshape(NDT, 128).T)  # [128, NDT]

    in_maps = []
    for a in range(B):
        # contraction zero-padded from 64 to 128 rows (exact same scores;
        # keeps the PE array fully row-occupied)
        kT_a = np.zeros((128, H, S), dtype=nbf)
        kT_a[:V] = kv[a].transpose(1, 2, 0).astype(nbf)
        v2_a = np.empty((S, H, V + 1), dtype=np.float32)
        v2_a[:, :, :V] = vv[a].transpose(0, 2, 1)  # [S, H, V]
        v2_a[:, :, V] = 1.0
        v2_t = np.ascontiguousarray(
            v2_a.reshape(NJT, 128, H, V + 1).transpose(1, 0, 2, 3)
        ).astype(nbf)  # [128, NJT, H, V+1]
        for ic in range(4):
            qT_c = np.zeros((128, H, IC), dtype=nbf)
            qT_c[:V] = qv[a, ic * IC : (ic + 1) * IC].transpose(1, 2, 0).astype(nbf)
            in_maps.append(
                {"qT": qT_c, "kT": kT_a, "v2": v2_t, "w2": w2_t, "bi": bi_t}
            )
    return in_maps


_PROGRAM_CACHE = []


def _get_program():
    if not _PROGRAM_CACHE:
        _PROGRAM_CACHE.append(build_program())
    return _PROGRAM_CACHE[0]


def run(inputs: dict, trace: bool = False):
    """Run on 8 cores; returns (full_output [B,S,D] f32, BassKernelResults)."""
    nc = _get_program()
    in_maps = make_core_inputs(**inputs)
    res = run_bass_kernel_spmd(nc, in_maps, core_ids=list(range(8)), trace=trace)
    out = np.empty((B, S, D), dtype=np.float32)
    for c in range(8):
        a, ic = c // 4, c % 4
        out[a, ic * IC : (ic + 1) * IC, :] = res.results[c]["outT"].T
    return out, res


def kernel(**inputs) -> np.ndarray:
    out, _ = run(inputs, trace=False)
    return out
